# revision 73
# baseline (speedup 1.0000x reference)
"""Trainium2 Bass kernel for nn_MultiHeadAttention_46093589021200.

Causal MHA: B=4, S=2048, E=1024, H=16, D=64, with the reference's
"no-transpose-back" reshape (b,h,s,d)->(b,s,e) before the output projection.

Sharding: pure head-parallel, 2 heads per core, zero collectives.
Because of the reshape quirk, output rows s' in [h*128,(h+1)*128) depend only
on head h, so each core produces two independent 128-row output bands per
batch.

Cost-model-driven structure (matmul cost = out_cols x pe_cycle; ldweights
free; PSUM bank-granular):
  - q,k: Wqkv_c^T @ x^T in head-major [col, s] layout, 8 K=128 chunks
    PSUM-accumulated, bias added on DVE during PSUM drain. v computed
    DIRECTLY in [s, d] layout (xt chunk stationary, Wv streaming) so no
    v transposes are needed; a strided DVE drain adds bias and writes the
    v2 chunk layout [v_h0|1|pad|v_h1|1|pad].
  - scoresT[k,q] per 128-k chunk, two heads row-packed (K=64 each).
  - exp on ACT (scale folded); causality = skipping k>q chunks + triangular
    fp16 mask multiply on diagonal 128x128 blocks (DVE).
  - PV FLIPPED: exp chunk [128k,128q] is the stationary, v_aug [128k,65]
    streams (ones column -> rowsums); att[q, d|sum] accumulates in PSUM.
    Normalize = DVE reciprocal + per-partition tensor_scalar_mul into a
    staging tile; one packed [128,128] DMA transpose per q-subchunk writes
    both heads' attnT into attn2 ([h0 d | h1 d] partitions x q cols).
  - o_proj w-PAIRED: per head a dup tile holds attnT on partitions 0-63
    and the same data shifted left 8 cols on partitions 64-127, so w and
    w+8 stack into K=128 matmuls (8 per (head, n2)); consumed through a
    stride-16 AP view (implements the reference's no-transpose reshape).
    Bias via DVE tensor_add during PSUM drain.
  - software pipelining: QKV(b+1) + o_proj(b-1) matmuls are drained as
    filler inside attention(b)'s inner loop so PE never waits on ACT exp.

NOTE: column-positioned matmuls (tile_position=(0,32j)) mis-execute on this
hardware path even though CoreSim accepts them - row-group packing only.
"""

import sys

if "/opt/trn_rl_repo" not in sys.path:
    sys.path.insert(0, "/opt/trn_rl_repo")

import numpy as np

B, S, E, H = 4, 2048, 1024, 16
D = E // H          # 64
NCORES = 8
HPC = H // NCORES   # heads per core = 2
COLS = 3 * HPC * D  # 384 qkv columns per core
SCALE = 1.0 / float(np.sqrt(D))
NQS = S // 128      # 16 q-subchunks per batch

_CACHE = {}


def _build_program(dbg=False):
    import concourse.bass as bass  # noqa: F401
    import concourse.tile as tile
    from concourse import bacc, mybir

    f16 = mybir.dt.float16
    f32 = mybir.dt.float32
    Exp = mybir.ActivationFunctionType.Exp

    nc = bacc.Bacc("TRN2", target_bir_lowering=False, debug=False)

    if dbg:
        dbg_att = nc.dram_tensor("dbg_att", [128, 260], f32, kind="ExternalOutput")
        dbg_ex = nc.dram_tensor("dbg_ex", [8, 128, 1024], f16, kind="ExternalOutput")

    xT = nc.dram_tensor("xT", [B, E, S], f16, kind="ExternalInput")
    wqkv = nc.dram_tensor("wqkv", [E, COLS], f16, kind="ExternalInput")
    bqkv = nc.dram_tensor("bqkv", [128, 3], f32, kind="ExternalInput")
    wo_pair = nc.dram_tensor("wo_pair", [8, 128, E], f16, kind="ExternalInput")
    bo_bc = nc.dram_tensor("bo_bc", [128, E], f32, kind="ExternalInput")
    trimask = nc.dram_tensor("trimask", [128, 128], f16, kind="ExternalInput")
    bvd = nc.dram_tensor("bvd", [128, 128], f32, kind="ExternalInput")
    out = nc.dram_tensor("out", [B, HPC, 128, E], f32, kind="ExternalOutput")

    with tile.TileContext(nc) as tc:
        with (
            tc.tile_pool(name="const", bufs=1) as cp,
            tc.tile_pool(name="sb", bufs=2) as sb,
            tc.tile_pool(name="sb3", bufs=3) as sb3,
            tc.tile_pool(name="ps", bufs=2, space="PSUM") as ps,
        ):
            # ---- tiles ----
            wqkv_sb = cp.tile([128, 8 * COLS], f16, tag="wqkv")
            bqkv_sb = cp.tile([128, 3], f32, tag="bqkv")
            trimask_sb = cp.tile([128, 128], f16, tag="trimask")
            wo_sb = cp.tile([128, 8 * E], f16, tag="wo")
            bo_sb = cp.tile([128, E], f32, tag="bo")
            bvd_sb = cp.tile([128, 128], f32, tag="bvd")
            # manual double-buffers (persistent; avoids tag-rotation WAR
            # stalls on the DMA rings)
            xts = [cp.tile([128, 8 * S], f16, tag=f"xt{i}", name=f"xt{i}")
                   for i in range(2)]
            v2s = [cp.tile([128, 160 * NQS], f16, tag=f"v2{i}", name=f"v2{i}")
                   for i in range(2)]
            dups = [[cp.tile([128, S], f16, tag=f"dup{i}_{h}",
                             name=f"dup{i}_{h}") for h in range(2)]
                    for i in range(3)]

            state = {}

            def xt_items(b, split_first=False, ring=None):
                """Closures issuing xt(b) loads as two big DMAs (wait-free:
                manual buffers, readers long done). Prologue loads ride SP;
                steady-state loads ride the otherwise-DMA-free ACT queue so
                qkv's xt-read sem thresholds never include transposes."""
                eng = ring if ring is not None else nc.sync
                xt_sb = xts[b % 2]
                xtv = xt_sb.rearrange("p (ec s) -> p ec s", ec=8)
                xt_dram = xT.ap()[b].rearrange("(ec p) s -> p ec s", p=128)
                state[("xt", b)] = xt_sb
                items = []
                if split_first:
                    # s-block-major: QKV group n can start once block n lands
                    for blk in range(4):
                        items.append(lambda blk=blk: eng.dma_start(
                            xtv[:, :, blk * 512 : (blk + 1) * 512],
                            xt_dram[:, :, blk * 512 : (blk + 1) * 512]))
                else:
                    items.append(lambda: eng.dma_start(
                        xtv[:, 0:4], xt_dram[:, 0:4]))
                    items.append(lambda: eng.dma_start(
                        xtv[:, 4:8], xt_dram[:, 4:8]))
                return items

            def qkv_work(b):
                """Closures computing qkvT2(b) + v2(b). Needs xt(b) issued."""
                qkvT2 = sb.tile([128, 2 * S], f16, tag="qkvT2", name=f"qkvT2_{b}")
                v2 = v2s[b % 2]
                state[("qkvT2", b)] = qkvT2
                state[("v2", b)] = v2
                blocks = []
                xt_sb = state[("xt", b)]
                for n in range(4):          # 512-wide s blocks
                    items = []
                    blocks.append(items)
                    for m in range(2):      # q, k column groups
                        pq = ps.tile([128, 512], f32, tag="acc", bufs=2,
                                     name=f"pq{b}_{n}_{m}")

                        def mk_mm(pq=pq, n=n, m=m, ecs=None):
                            def f():
                                for ec in ecs:
                                    nc.tensor.matmul(
                                        pq,
                                        wqkv_sb[:, ec * COLS + m * 128 :
                                                ec * COLS + (m + 1) * 128],
                                        xt_sb[:, ec * S + n * 512 :
                                              ec * S + (n + 1) * 512],
                                        start=(ec == 0),
                                        stop=(ec == 7),
                                    )
                            return f

                        for g in range(4):
                            items.append(mk_mm(ecs=(2 * g, 2 * g + 1)))

                        def mk_bias(pq=pq, n=n, m=m):
                            def f():
                                nc.vector.tensor_scalar_add(
                                    qkvT2[:, m * S + n * 512 :
                                          m * S + (n + 1) * 512],
                                    pq,
                                    bqkv_sb[:, m : m + 1],
                                )
                            return f

                        items.append(mk_bias())
                    # v for this block, computed directly in [s, d] layout:
                    # xt chunk stationary, Wv streams -> out [128 s, 128 d2]
                    for st in range(4 * n, 4 * n + 4):
                        pv = ps.tile([128, 128], f32, tag="acc", bufs=2,
                                     name=f"pv{b}_{st}")

                        def mk_vmm(pv=pv, st=st, ecs=None):
                            def f():
                                for ec in ecs:
                                    nc.tensor.matmul(
                                        pv,
                                        xt_sb[:, ec * S + st * 128 :
                                              ec * S + (st + 1) * 128],
                                        wqkv_sb[:, ec * COLS + 256 :
                                                ec * COLS + 384],
                                        start=(ec == 0),
                                        stop=(ec == 7),
                                    )
                            return f

                        for g in range(4):
                            items.append(mk_vmm(ecs=(2 * g, 2 * g + 1)))

                        def mk_vdrain(pv=pv, st=st):
                            def f():
                                dst = v2[:, st * 160 : (st + 1) * 160]
                                dst = dst.rearrange(
                                    "p (g c) -> p g c", c=80)[:, :, 0:64]
                                nc.vector.tensor_add(
                                    dst,
                                    pv.rearrange("p (g c) -> p g c", c=64),
                                    bvd_sb.rearrange(
                                        "p (g c) -> p g c", c=64),
                                )
                            return f

                        items.append(mk_vdrain())
                return blocks

            def oproj_work(b):
                """Closures for o_proj(b). Needs dup(b) tiles complete."""
                items = []
                for h in range(2):
                    out_sb = sb.tile([128, E], f32, tag=f"outsb{h}",
                                     name=f"out{h}_sb_{b}")
                    attv = dups[b % 3][h].rearrange("p (u w) -> p w u", w=16)
                    for n2 in range(2):
                        po = ps.tile([128, 512], f32, tag="acc", bufs=2,
                                     name=f"po{b}_{h}_{n2}")

                        def mk_mm(po=po, attv=attv, n2=n2, ws=None):
                            def f():
                                for w in ws:
                                    nc.tensor.matmul(
                                        po,
                                        attv[:, w : w + 1, :],
                                        wo_sb[:, w * E + n2 * 512 :
                                              w * E + (n2 + 1) * 512],
                                        start=(w == 0),
                                        stop=(w == 7),
                                    )
                            return f

                        for g in range(4):
                            items.append(mk_mm(ws=(2 * g, 2 * g + 1)))

                        def mk_bias(po=po, out_sb=out_sb, n2=n2):
                            def f():
                                nc.vector.tensor_add(
                                    out_sb[:, n2 * 512 : (n2 + 1) * 512],
                                    po,
                                    bo_sb[:, n2 * 512 : (n2 + 1) * 512],
                                )
                            return f

                        items.append(mk_bias())

                    def mk_out(b=b, h=h, out_sb=out_sb):
                        def f():
                            nc.gpsimd.dma_start(out.ap()[b, h], out_sb)
                        return f

                    items.append(mk_out())
                return items

            def attention(b, filler, early=(), delay=None):
                """Attention for batch b, draining `filler` closures evenly."""
                qkvT2 = state[("qkvT2", b)]
                v2 = state[("v2", b)]
                attn2 = sb.tile([128, S], f16, tag="attn2", name=f"attn2_{b}")
                state[("attn2", b)] = attn2

                fill_i = 0
                n_iter = sum(4 * gq + 4 for gq in range(4))
                delay = delay if delay is not None else 22  # let xt(b+1) land before qkv fillers hit PE's FIFO
                it = 0

                def drain(it):
                    nonlocal fill_i
                    target = len(filler) * max(0, it - delay) // (n_iter - delay)
                    while fill_i < min(target, len(filler)):
                        filler[fill_i]()
                        fill_i += 1

                def issue_sc(gq, kj):
                    q_lo = max(gq * 512, kj * 128)
                    W = gq * 512 + 512 - q_lo
                    qo = q_lo - gq * 512
                    sc = ps.tile([128, 1024], f32, tag="sc", bufs=2,
                                 name=f"sc{b}_{gq}_{kj}")
                    for h in range(2):
                        nc.tensor.matmul(
                            sc[:, h * 512 + qo : h * 512 + qo + W],
                            qkvT2[h * 64 : (h + 1) * 64,
                                  S + kj * 128 : S + (kj + 1) * 128],
                            qkvT2[h * 64 : (h + 1) * 64, q_lo : q_lo + W],
                            start=True,
                            stop=True,
                            tile_position=(h * 64, 0),
                        )
                    ex = sb3.tile([128, 1024], f16, tag="ex",
                                  name=f"ex{b}_{gq}_{kj}")
                    nc.scalar.activation(
                        ex.rearrange("p (h q) -> p h q", h=2)[:, :, qo : qo + W],
                        sc.rearrange("p (h q) -> p h q", h=2)[:, :, qo : qo + W],
                        Exp,
                        scale=SCALE,
                    )
                    if kj >= 4 * gq:  # diagonal chunk: zero out k > q
                        for h in range(2):
                            nc.vector.tensor_mul(
                                ex[:, h * 512 + qo : h * 512 + qo + 128],
                                ex[:, h * 512 + qo : h * 512 + qo + 128],
                                trimask_sb,
                            )
                    return ex

                pending_tr = []
                for gq in range(4):
                    njk = 4 * gq + 4
                    att = [
                        ps.tile([128, 260], f32, tag=f"att{h}", bufs=1,
                                name=f"att{h}_{b}_{gq}")
                        for h in range(2)
                    ]
                    exs = [None] * njk
                    exs[0] = issue_sc(gq, 0)
                    for kj in range(njk):
                        if kj + 1 < njk:
                            exs[kj + 1] = issue_sc(gq, kj + 1)
                        # ready fillers BEFORE pv(kj): pv's ldweights waits on
                        # exp(kj) and would head-block them in PE's FIFO
                        it += 1
                        drain(it)
                        # PV for kj (flipped: exp stationary, v_aug streams)
                        ex = exs[kj]
                        i_min = max(0, kj - 4 * gq)
                        for h in range(2):
                            for i in range(i_min, 4):
                                # start=True clears has_written for the WHOLE
                                # bank on this hw path: only the bank's first
                                # MM may set it; later kj==0 subchunk writes
                                # land on has_written=0 -> overwrite.
                                nc.tensor.matmul(
                                    att[h][:, i * 65 : (i + 1) * 65],
                                    ex[:, h * 512 + i * 128 :
                                       h * 512 + (i + 1) * 128],
                                    v2[:, kj * 160 + h * 80 :
                                       kj * 160 + h * 80 + 65],
                                    start=(kj == 0 and i == 0),
                                    stop=(kj == 4 * gq + i),
                                )
                        exs[kj] = None
                        if it == 3:
                            for e in early:
                                e()
                        if kj == njk - 1 and pending_tr:
                            for tr in pending_tr:
                                tr()
                            pending_tr = []
                        if dbg and b == 0 and gq == 1:
                            nc.sync.dma_start(dbg_ex.ap()[kj], ex)
                    if dbg and b == 0 and gq == 1:
                        datt = sb.tile([128, 260], f32, tag="datt", name="datt")
                        nc.vector.tensor_copy(datt, att[0])
                        nc.sync.dma_start(dbg_att.ap(), datt)
                    # normalize + packed transpose into attn2
                    stg = sb.tile([128, 512], f16, tag="stg", bufs=3,
                                  name=f"stg{b}_{gq}")
                    for h in range(2):
                        av = att[h].rearrange("p (i c) -> p i c", c=65)
                        rr = sb.tile([128, 4], f32, tag=f"rr{h}",
                                     name=f"rr{h}_{b}_{gq}")
                        nc.vector.reciprocal(
                            rr.rearrange("p (i c) -> p i c", c=1),
                            av[:, :, 64:65],
                        )
                        for i in range(4):
                            nc.vector.tensor_scalar_mul(
                                stg[:, i * 128 + h * 64 : i * 128 + h * 64 + 64],
                                av[:, i, 0:64],
                                rr[:, i : i + 1],
                            )
                    for i in range(4):
                        def mk_tr(gq=gq, i=i, stg=stg):
                            def f():
                                nc.sync.dma_start(
                                    attn2[:, (gq * 4 + i) * 128 :
                                          (gq * 4 + i + 1) * 128],
                                    stg[:, i * 128 : (i + 1) * 128],
                                    transpose=True,
                                )
                            return f
                        pending_tr.append(mk_tr())
                for tr in pending_tr:
                    tr()
                # dup tiles for o_proj(b): same head's attnT on both partition
                # halves, upper half shifted left 8 cols (w/w+8 pairing).
                # SP queue, right behind the attn2 transposes they read.
                for h in range(2):
                    dup = dups[b % 3][h]
                    nc.sync.dma_start(
                        dup[0:64, :], attn2[h * 64 : (h + 1) * 64, :])
                    nc.sync.dma_start(
                        dup[64:128, 0 : S - 8],
                        attn2[h * 64 : (h + 1) * 64, 8:S])
                while fill_i < len(filler):
                    filler[fill_i]()
                    fill_i += 1

            # ================= main schedule =================
            # xt(0) first so its DMA transfers lead the serial DMA queue;
            # wo (4MB) deliberately later - o_proj(0) needs it only during
            # attention(1).
            xt0 = xt_items(0, split_first=True)
            xt0[0]()
            nc.sync.dma_start(
                wqkv_sb.rearrange("p (ec c) -> p ec c", ec=8),
                wqkv.ap().rearrange("(ec p) c -> p ec c", p=128),
            )
            for item in xt0[1:]:
                item()
            nc.sync.dma_start(bqkv_sb, bqkv.ap())
            nc.sync.dma_start(trimask_sb, trimask.ap())
            nc.sync.dma_start(bvd_sb, bvd.ap())
            for item in xt_items(1):
                item()
            for i in range(2):
                v2v = v2s[i].rearrange("p (c t) -> p c t", t=160)
                nc.gpsimd.memset(v2v[:, :, 64:65], 1.0)
                nc.gpsimd.memset(v2v[:, :, 144:145], 1.0)
            nc.sync.dma_start(
                wo_sb.rearrange("p (w c) -> p w c", w=8),
                wo_pair.ap().rearrange("w p c -> p w c"),
            )
            nc.sync.dma_start(bo_sb, bo_bc.ap())
            for blk in qkv_work(0):
                for item in blk:
                    item()
            for b in range(B):
                filler = []
                early = []
                if b + 2 < B:
                    for item in xt_items(b + 2):
                        item()
                if b == B - 1:
                    # all deferred o_proj lands here: attention(3) is the only
                    # window without next-batch QKV filler (ACT-bound else)
                    for j in range(B - 1):
                        filler += oproj_work(j)
                if b + 1 < B:
                    for blk in qkv_work(b + 1):
                        filler += blk
                attention(b, filler, early,
                          delay=2 if b == B - 1 else None)
            for item in oproj_work(B - 1):
                item()

    nc.compile()
    return nc


def _get_program(dbg=False):
    key = ("nc", dbg)
    if key not in _CACHE:
        _CACHE[key] = _build_program(dbg)
    return _CACHE[key]


def _host_inputs(x, Wqkv, bqkv, Wo, bo):
    """Build per-core input maps (host-side layout prep: cast/slice/transpose)."""
    xT = np.ascontiguousarray(x.transpose(0, 2, 1)).astype(np.float16)

    wo16 = Wo.astype(np.float16)
    wo_pair = np.empty((8, 128, E), np.float16)
    for w in range(8):
        wo_pair[w, 0:64] = wo16[w * 64 : (w + 1) * 64]
        wo_pair[w, 64:128] = wo16[(w + 8) * 64 : (w + 9) * 64]

    bo_bc = np.broadcast_to(bo.astype(np.float32), (128, E)).copy()

    k_idx = np.arange(128)[:, None]
    q_idx = np.arange(128)[None, :]
    trimask = (k_idx <= q_idx).astype(np.float16)

    in_maps = []
    for c in range(NCORES):
        cols = []
        for off in (0, 64, 128):  # q, k, v
            for h in (HPC * c, HPC * c + 1):
                cols.extend(range(h * 3 * D + off, h * 3 * D + off + 64))
        cols = np.asarray(cols)
        in_maps.append(
            {
                "xT": xT,
                "wqkv": np.ascontiguousarray(Wqkv[:, cols]).astype(np.float16),
                "bqkv": np.ascontiguousarray(
                    bqkv[cols].reshape(3, 128).T
                ).astype(np.float32),
                "wo_pair": wo_pair,
                "bo_bc": bo_bc,
                "trimask": trimask,
                "bvd": np.broadcast_to(
                    bqkv[cols][256:384].astype(np.float32), (128, 128)
                ).copy(),
            }
        )
    return in_maps


def kernel(x, mask, Wqkv, bqkv, Wo, bo, _n_cores=NCORES, _trace=False, _dbg=False):
    """Full-input, full-output MHA. `mask` is the causal tril mask (hardcoded)."""
    from concourse.bass_utils import run_bass_kernel_spmd

    nc = _get_program(_dbg)
    in_maps = _host_inputs(
        np.asarray(x), np.asarray(Wqkv), np.asarray(bqkv), np.asarray(Wo), np.asarray(bo)
    )[:_n_cores]
    res = run_bass_kernel_spmd(
        nc, in_maps, core_ids=list(range(_n_cores)), trace=_trace
    )
    out_full = np.zeros((B, S, E), np.float32)
    for c in range(_n_cores):
        o = res.results[c]["out"]  # [B, HPC, 128, E]
        for h in range(HPC):
            g = HPC * c + h
            out_full[:, g * 128 : (g + 1) * 128, :] = o[:, h]
    _CACHE["last_results"] = res
    return out_full


# revision 79
# speedup vs baseline: 1.0089x; 1.0089x over previous
"""Trainium2 Bass kernel for nn_MultiHeadAttention_46093589021200.

Causal MHA: B=4, S=2048, E=1024, H=16, D=64, with the reference's
"no-transpose-back" reshape (b,h,s,d)->(b,s,e) before the output projection.

Sharding: pure head-parallel, 2 heads per core, zero collectives.
Because of the reshape quirk, output rows s' in [h*128,(h+1)*128) depend only
on head h, so each core produces two independent 128-row output bands per
batch.

Cost-model-driven structure (matmul cost = out_cols x pe_cycle; ldweights
free; PSUM bank-granular):
  - q,k: Wqkv_c^T @ x^T in head-major [col, s] layout, 8 K=128 chunks
    PSUM-accumulated, bias added on DVE during PSUM drain. v computed
    DIRECTLY in [s, d] layout (xt chunk stationary, Wv streaming) so no
    v transposes are needed; a strided DVE drain adds bias and writes the
    v2 chunk layout [v_h0|1|pad|v_h1|1|pad].
  - scoresT[k,q] per 128-k chunk, two heads row-packed (K=64 each).
  - exp on ACT (scale folded); causality = skipping k>q chunks + triangular
    fp16 mask multiply on diagonal 128x128 blocks (DVE).
  - PV FLIPPED: exp chunk [128k,128q] is the stationary, v_aug [128k,65]
    streams (ones column -> rowsums); att[q, d|sum] accumulates in PSUM.
    Normalize = DVE reciprocal + per-partition tensor_scalar_mul into a
    staging tile; one packed [128,128] DMA transpose per q-subchunk writes
    both heads' attnT into attn2 ([h0 d | h1 d] partitions x q cols).
  - o_proj w-PAIRED: per head a dup tile holds attnT on partitions 0-63
    and the same data shifted left 8 cols on partitions 64-127, so w and
    w+8 stack into K=128 matmuls (8 per (head, n2)); consumed through a
    stride-16 AP view (implements the reference's no-transpose reshape).
    Bias via DVE tensor_add during PSUM drain.
  - software pipelining: QKV(b+1) + o_proj(b-1) matmuls are drained as
    filler inside attention(b)'s inner loop so PE never waits on ACT exp.

NOTE: column-positioned matmuls (tile_position=(0,32j)) mis-execute on this
hardware path even though CoreSim accepts them - row-group packing only.
"""

import sys

if "/opt/trn_rl_repo" not in sys.path:
    sys.path.insert(0, "/opt/trn_rl_repo")

import numpy as np

B, S, E, H = 4, 2048, 1024, 16
D = E // H          # 64
NCORES = 8
HPC = H // NCORES   # heads per core = 2
COLS = 3 * HPC * D  # 384 qkv columns per core
SCALE = 1.0 / float(np.sqrt(D))
NQS = S // 128      # 16 q-subchunks per batch

_CACHE = {}


def _build_program(dbg=False):
    import concourse.bass as bass  # noqa: F401
    import concourse.tile as tile
    from concourse import bacc, mybir

    f16 = mybir.dt.float16
    f32 = mybir.dt.float32
    Exp = mybir.ActivationFunctionType.Exp

    nc = bacc.Bacc("TRN2", target_bir_lowering=False, debug=False)

    if dbg:
        dbg_att = nc.dram_tensor("dbg_att", [128, 260], f32, kind="ExternalOutput")
        dbg_ex = nc.dram_tensor("dbg_ex", [8, 128, 1024], f16, kind="ExternalOutput")

    xT = nc.dram_tensor("xT", [B, E, S], f16, kind="ExternalInput")
    wqkv = nc.dram_tensor("wqkv", [E, COLS], f16, kind="ExternalInput")
    bqkv = nc.dram_tensor("bqkv", [128, 3], f32, kind="ExternalInput")
    wo_pair = nc.dram_tensor("wo_pair", [8, 128, E], f16, kind="ExternalInput")
    bo_bc = nc.dram_tensor("bo_bc", [128, E], f32, kind="ExternalInput")
    trimask = nc.dram_tensor("trimask", [128, 128], f16, kind="ExternalInput")
    bvd = nc.dram_tensor("bvd", [128, 128], f32, kind="ExternalInput")
    out = nc.dram_tensor("out", [B, HPC, 128, E], f32, kind="ExternalOutput")

    with tile.TileContext(nc) as tc:
        with (
            tc.tile_pool(name="const", bufs=1) as cp,
            tc.tile_pool(name="sb", bufs=2) as sb,
            tc.tile_pool(name="sb3", bufs=3) as sb3,
            tc.tile_pool(name="ps", bufs=2, space="PSUM") as ps,
        ):
            # ---- tiles ----
            wqkv_sb = cp.tile([128, 8 * COLS], f16, tag="wqkv")
            bqkv_sb = cp.tile([128, 3], f32, tag="bqkv")
            trimask_sb = cp.tile([128, 128], f16, tag="trimask")
            wo_sb = cp.tile([128, 8 * E], f16, tag="wo")
            bo_sb = cp.tile([128, E], f32, tag="bo")
            bvd_sb = cp.tile([128, 128], f32, tag="bvd")
            # manual double-buffers (persistent; avoids tag-rotation WAR
            # stalls on the DMA rings)
            xts = [cp.tile([128, 8 * S], f16, tag=f"xt{i}", name=f"xt{i}")
                   for i in range(2)]
            v2s = [cp.tile([128, 160 * NQS], f16, tag=f"v2{i}", name=f"v2{i}")
                   for i in range(2)]
            dups = [[cp.tile([128, S], f16, tag=f"dup{i}_{h}",
                             name=f"dup{i}_{h}") for h in range(2)]
                    for i in range(3)]

            state = {}

            def xt_items(b, split_first=False, ring=None):
                """Closures issuing xt(b) loads as two big DMAs (wait-free:
                manual buffers, readers long done). Prologue loads ride SP;
                steady-state loads ride the otherwise-DMA-free ACT queue so
                qkv's xt-read sem thresholds never include transposes."""
                eng = ring if ring is not None else nc.sync
                xt_sb = xts[b % 2]
                xtv = xt_sb.rearrange("p (ec s) -> p ec s", ec=8)
                xt_dram = xT.ap()[b].rearrange("(ec p) s -> p ec s", p=128)
                state[("xt", b)] = xt_sb
                items = []
                if split_first:
                    # s-block-major: QKV group n can start once block n lands
                    for blk in range(4):
                        items.append(lambda blk=blk: eng.dma_start(
                            xtv[:, :, blk * 512 : (blk + 1) * 512],
                            xt_dram[:, :, blk * 512 : (blk + 1) * 512]))
                else:
                    items.append(lambda: eng.dma_start(
                        xtv[:, 0:4], xt_dram[:, 0:4]))
                    items.append(lambda: eng.dma_start(
                        xtv[:, 4:8], xt_dram[:, 4:8]))
                return items

            def qkv_work(b):
                """Closures computing qkvT2(b) + v2(b). Needs xt(b) issued."""
                qkvT2 = sb.tile([128, 2 * S], f16, tag="qkvT2", name=f"qkvT2_{b}")
                v2 = v2s[b % 2]
                state[("qkvT2", b)] = qkvT2
                state[("v2", b)] = v2
                blocks = []
                xt_sb = state[("xt", b)]
                for n in range(4):          # 512-wide s blocks
                    items = []
                    blocks.append(items)
                    for m in range(2):      # q, k column groups
                        pq = ps.tile([128, 512], f32, tag="acc", bufs=2,
                                     name=f"pq{b}_{n}_{m}")

                        def mk_mm(pq=pq, n=n, m=m, ecs=None):
                            def f():
                                for ec in ecs:
                                    nc.tensor.matmul(
                                        pq,
                                        wqkv_sb[:, ec * COLS + m * 128 :
                                                ec * COLS + (m + 1) * 128],
                                        xt_sb[:, ec * S + n * 512 :
                                              ec * S + (n + 1) * 512],
                                        start=(ec == 0),
                                        stop=(ec == 7),
                                    )
                            return f

                        for g in range(4):
                            items.append(mk_mm(ecs=(2 * g, 2 * g + 1)))

                        def mk_bias(pq=pq, n=n, m=m):
                            def f():
                                nc.vector.tensor_scalar_add(
                                    qkvT2[:, m * S + n * 512 :
                                          m * S + (n + 1) * 512],
                                    pq,
                                    bqkv_sb[:, m : m + 1],
                                )
                            return f

                        items.append(mk_bias())
                    # v for this block, computed directly in [s, d] layout:
                    # xt chunk stationary, Wv streams -> out [128 s, 128 d2]
                    for st in range(4 * n, 4 * n + 4):
                        pv = ps.tile([128, 128], f32, tag="acc", bufs=2,
                                     name=f"pv{b}_{st}")

                        def mk_vmm(pv=pv, st=st, ecs=None):
                            def f():
                                for ec in ecs:
                                    nc.tensor.matmul(
                                        pv,
                                        xt_sb[:, ec * S + st * 128 :
                                              ec * S + (st + 1) * 128],
                                        wqkv_sb[:, ec * COLS + 256 :
                                                ec * COLS + 384],
                                        start=(ec == 0),
                                        stop=(ec == 7),
                                    )
                            return f

                        for g in range(4):
                            items.append(mk_vmm(ecs=(2 * g, 2 * g + 1)))

                        def mk_vdrain(pv=pv, st=st):
                            def f():
                                dst = v2[:, st * 160 : (st + 1) * 160]
                                dst = dst.rearrange(
                                    "p (g c) -> p g c", c=80)[:, :, 0:64]
                                nc.vector.tensor_add(
                                    dst,
                                    pv.rearrange("p (g c) -> p g c", c=64),
                                    bvd_sb.rearrange(
                                        "p (g c) -> p g c", c=64),
                                )
                            return f

                        items.append(mk_vdrain())
                return blocks

            def oproj_work(b):
                """Closures for o_proj(b). Needs dup(b) tiles complete."""
                items = []
                for h in range(2):
                    out_sb = sb.tile([128, E], f32, tag=f"outsb{h}",
                                     name=f"out{h}_sb_{b}")
                    attv = dups[b % 3][h].rearrange("p (u w) -> p w u", w=16)
                    for n2 in range(2):
                        po = ps.tile([128, 512], f32, tag="acc", bufs=2,
                                     name=f"po{b}_{h}_{n2}")

                        def mk_mm(po=po, attv=attv, n2=n2, ws=None):
                            def f():
                                for w in ws:
                                    nc.tensor.matmul(
                                        po,
                                        attv[:, w : w + 1, :],
                                        wo_sb[:, w * E + n2 * 512 :
                                              w * E + (n2 + 1) * 512],
                                        start=(w == 0),
                                        stop=(w == 7),
                                    )
                            return f

                        for g in range(4):
                            items.append(mk_mm(ws=(2 * g, 2 * g + 1)))

                        def mk_bias(po=po, out_sb=out_sb, n2=n2):
                            def f():
                                nc.vector.tensor_add(
                                    out_sb[:, n2 * 512 : (n2 + 1) * 512],
                                    po,
                                    bo_sb[:, n2 * 512 : (n2 + 1) * 512],
                                )
                            return f

                        items.append(mk_bias())

                    def mk_out(b=b, h=h, out_sb=out_sb):
                        def f():
                            nc.gpsimd.dma_start(out.ap()[b, h], out_sb)
                        return f

                    items.append(mk_out())
                return items

            def attention(b, filler, early=(), delay=None, n_fin=None):
                """Attention for batch b, draining `filler` closures evenly."""
                qkvT2 = state[("qkvT2", b)]
                v2 = state[("v2", b)]
                attn2 = sb.tile([128, S], f16, tag="attn2", name=f"attn2_{b}")
                state[("attn2", b)] = attn2

                fill_i = 0
                n_iter = sum(4 * gq + 4 for gq in range(4))
                delay = delay if delay is not None else 22  # let xt(b+1) land before qkv fillers hit PE's FIFO
                it = 0

                fin = n_fin if n_fin is not None else n_iter

                def drain(it):
                    nonlocal fill_i
                    target = len(filler) * max(0, it - delay) // (fin - delay)
                    while fill_i < min(target, len(filler)):
                        filler[fill_i]()
                        fill_i += 1

                def issue_sc(gq, kj):
                    q_lo = max(gq * 512, kj * 128)
                    W = gq * 512 + 512 - q_lo
                    qo = q_lo - gq * 512
                    sc = ps.tile([128, 1024], f32, tag="sc", bufs=2,
                                 name=f"sc{b}_{gq}_{kj}")
                    for h in range(2):
                        nc.tensor.matmul(
                            sc[:, h * 512 + qo : h * 512 + qo + W],
                            qkvT2[h * 64 : (h + 1) * 64,
                                  S + kj * 128 : S + (kj + 1) * 128],
                            qkvT2[h * 64 : (h + 1) * 64, q_lo : q_lo + W],
                            start=True,
                            stop=True,
                            tile_position=(h * 64, 0),
                        )
                    ex = sb3.tile([128, 1024], f16, tag="ex",
                                  name=f"ex{b}_{gq}_{kj}")
                    nc.scalar.activation(
                        ex.rearrange("p (h q) -> p h q", h=2)[:, :, qo : qo + W],
                        sc.rearrange("p (h q) -> p h q", h=2)[:, :, qo : qo + W],
                        Exp,
                        scale=SCALE,
                    )
                    if kj >= 4 * gq:  # diagonal chunk: zero out k > q
                        for h in range(2):
                            nc.vector.tensor_mul(
                                ex[:, h * 512 + qo : h * 512 + qo + 128],
                                ex[:, h * 512 + qo : h * 512 + qo + 128],
                                trimask_sb,
                            )
                    return ex

                pending_tr = []
                for gq in range(4):
                    njk = 4 * gq + 4
                    att = [
                        ps.tile([128, 260], f32, tag=f"att{h}", bufs=1,
                                name=f"att{h}_{b}_{gq}")
                        for h in range(2)
                    ]
                    exs = [None] * njk
                    exs[0] = issue_sc(gq, 0)
                    for kj in range(njk):
                        if kj + 1 < njk:
                            exs[kj + 1] = issue_sc(gq, kj + 1)
                        # ready fillers BEFORE pv(kj): pv's ldweights waits on
                        # exp(kj) and would head-block them in PE's FIFO
                        it += 1
                        drain(it)
                        # PV for kj (flipped: exp stationary, v_aug streams)
                        ex = exs[kj]
                        i_min = max(0, kj - 4 * gq)
                        for h in range(2):
                            for i in range(i_min, 4):
                                # start=True clears has_written for the WHOLE
                                # bank on this hw path: only the bank's first
                                # MM may set it; later kj==0 subchunk writes
                                # land on has_written=0 -> overwrite.
                                nc.tensor.matmul(
                                    att[h][:, i * 65 : (i + 1) * 65],
                                    ex[:, h * 512 + i * 128 :
                                       h * 512 + (i + 1) * 128],
                                    v2[:, kj * 160 + h * 80 :
                                       kj * 160 + h * 80 + 65],
                                    start=(kj == 0 and i == 0),
                                    stop=(kj == 4 * gq + i),
                                )
                        exs[kj] = None
                        if it == 3:
                            for e in early:
                                e()
                        if kj == njk - 1 and pending_tr:
                            for tr in pending_tr:
                                tr()
                            pending_tr = []
                        if dbg and b == 0 and gq == 1:
                            nc.sync.dma_start(dbg_ex.ap()[kj], ex)
                    if dbg and b == 0 and gq == 1:
                        datt = sb.tile([128, 260], f32, tag="datt", name="datt")
                        nc.vector.tensor_copy(datt, att[0])
                        nc.sync.dma_start(dbg_att.ap(), datt)
                    # normalize + packed transpose into attn2
                    stg = sb.tile([128, 512], f16, tag="stg", bufs=3,
                                  name=f"stg{b}_{gq}")
                    for h in range(2):
                        av = att[h].rearrange("p (i c) -> p i c", c=65)
                        rr = sb.tile([128, 4], f32, tag=f"rr{h}",
                                     name=f"rr{h}_{b}_{gq}")
                        nc.vector.reciprocal(
                            rr.rearrange("p (i c) -> p i c", c=1),
                            av[:, :, 64:65],
                        )
                        for i in range(4):
                            nc.vector.tensor_scalar_mul(
                                stg[:, i * 128 + h * 64 : i * 128 + h * 64 + 64],
                                av[:, i, 0:64],
                                rr[:, i : i + 1],
                            )
                    for i in range(4):
                        def mk_tr(gq=gq, i=i, stg=stg):
                            def f():
                                nc.sync.dma_start(
                                    attn2[:, (gq * 4 + i) * 128 :
                                          (gq * 4 + i + 1) * 128],
                                    stg[:, i * 128 : (i + 1) * 128],
                                    transpose=True,
                                )
                            return f
                        pending_tr.append(mk_tr())
                # leftover fillers FIRST: anything issued after the final
                # transposes/dup copies inherits their position-based DMA
                # thresholds and waits for the whole end-of-batch tail
                while fill_i < len(filler):
                    filler[fill_i]()
                    fill_i += 1
                for tr in pending_tr:
                    tr()
                # dup tiles for o_proj(b): same head's attnT on both partition
                # halves, upper half shifted left 8 cols (w/w+8 pairing).
                # SP queue, right behind the attn2 transposes they read.
                for h in range(2):
                    dup = dups[b % 3][h]
                    nc.sync.dma_start(
                        dup[0:64, :], attn2[h * 64 : (h + 1) * 64, :])
                    nc.sync.dma_start(
                        dup[64:128, 0 : S - 8],
                        attn2[h * 64 : (h + 1) * 64, 8:S])

            # ================= main schedule =================
            # xt(0) first so its DMA transfers lead the serial DMA queue;
            # wo (4MB) deliberately later - o_proj(0) needs it only during
            # attention(1).
            xt0 = xt_items(0, split_first=True)
            xt0[0]()
            nc.sync.dma_start(
                wqkv_sb.rearrange("p (ec c) -> p ec c", ec=8),
                wqkv.ap().rearrange("(ec p) c -> p ec c", p=128),
            )
            for item in xt0[1:]:
                item()
            nc.sync.dma_start(bqkv_sb, bqkv.ap())
            nc.sync.dma_start(trimask_sb, trimask.ap())
            nc.sync.dma_start(bvd_sb, bvd.ap())
            for item in xt_items(1):
                item()
            for i in range(2):
                v2v = v2s[i].rearrange("p (c t) -> p c t", t=160)
                nc.gpsimd.memset(v2v[:, :, 64:65], 1.0)
                nc.gpsimd.memset(v2v[:, :, 144:145], 1.0)
            nc.sync.dma_start(
                wo_sb.rearrange("p (w c) -> p w c", w=8),
                wo_pair.ap().rearrange("w p c -> p w c"),
            )
            nc.sync.dma_start(bo_sb, bo_bc.ap())
            for blk in qkv_work(0):
                for item in blk:
                    item()
            for b in range(B):
                filler = []
                early = []
                if b + 2 < B:
                    for item in xt_items(b + 2):
                        item()
                if b == B - 1:
                    # all deferred o_proj lands here: attention(3) is the only
                    # window without next-batch QKV filler (ACT-bound else)
                    for j in range(B - 1):
                        filler += oproj_work(j)
                if b + 1 < B:
                    for blk in qkv_work(b + 1):
                        filler += blk
                attention(b, filler, early,
                          delay=2 if b == B - 1 else None,
                          n_fin=36 if b == B - 1 else None)
            for item in oproj_work(B - 1):
                item()

    nc.compile()
    return nc


def _get_program(dbg=False):
    key = ("nc", dbg)
    if key not in _CACHE:
        _CACHE[key] = _build_program(dbg)
    return _CACHE[key]


def _host_inputs(x, Wqkv, bqkv, Wo, bo):
    """Build per-core input maps (host-side layout prep: cast/slice/transpose)."""
    xT = np.ascontiguousarray(x.transpose(0, 2, 1)).astype(np.float16)

    wo16 = Wo.astype(np.float16)
    wo_pair = np.empty((8, 128, E), np.float16)
    for w in range(8):
        wo_pair[w, 0:64] = wo16[w * 64 : (w + 1) * 64]
        wo_pair[w, 64:128] = wo16[(w + 8) * 64 : (w + 9) * 64]

    bo_bc = np.broadcast_to(bo.astype(np.float32), (128, E)).copy()

    k_idx = np.arange(128)[:, None]
    q_idx = np.arange(128)[None, :]
    trimask = (k_idx <= q_idx).astype(np.float16)

    in_maps = []
    for c in range(NCORES):
        cols = []
        for off in (0, 64, 128):  # q, k, v
            for h in (HPC * c, HPC * c + 1):
                cols.extend(range(h * 3 * D + off, h * 3 * D + off + 64))
        cols = np.asarray(cols)
        in_maps.append(
            {
                "xT": xT,
                "wqkv": np.ascontiguousarray(Wqkv[:, cols]).astype(np.float16),
                "bqkv": np.ascontiguousarray(
                    bqkv[cols].reshape(3, 128).T
                ).astype(np.float32),
                "wo_pair": wo_pair,
                "bo_bc": bo_bc,
                "trimask": trimask,
                "bvd": np.broadcast_to(
                    bqkv[cols][256:384].astype(np.float32), (128, 128)
                ).copy(),
            }
        )
    return in_maps


def kernel(x, mask, Wqkv, bqkv, Wo, bo, _n_cores=NCORES, _trace=False, _dbg=False):
    """Full-input, full-output MHA. `mask` is the causal tril mask (hardcoded)."""
    from concourse.bass_utils import run_bass_kernel_spmd

    nc = _get_program(_dbg)
    in_maps = _host_inputs(
        np.asarray(x), np.asarray(Wqkv), np.asarray(bqkv), np.asarray(Wo), np.asarray(bo)
    )[:_n_cores]
    res = run_bass_kernel_spmd(
        nc, in_maps, core_ids=list(range(_n_cores)), trace=_trace
    )
    out_full = np.zeros((B, S, E), np.float32)
    for c in range(_n_cores):
        o = res.results[c]["out"]  # [B, HPC, 128, E]
        for h in range(HPC):
            g = HPC * c + h
            out_full[:, g * 128 : (g + 1) * 128, :] = o[:, h]
    _CACHE["last_results"] = res
    return out_full
